# revision 11
# baseline (speedup 1.0000x reference)
"""Trainium2 Bass kernel for CrossAttention (self-attention) nn module.

Reference computation (B=2, N=4096, D=512, H=8, DH=64):
    q, k, v = x@Wq, x@Wk, x@Wv          # [B, N, 512]
    per head: S = q k^T / sqrt(64); P = softmax(S); O = P v
    out = concat_heads(O) @ Wo + bo     # [B, N, 512]

Sharding: batch*head-pair across 8 cores. Core c handles batch c//4 and
head pair c%4 (heads 2p, 2p+1). Each core computes its two heads'
attention plus its partial output projection O_pair @ Wo[rows]; the host
sums the four partials per batch and adds the bias.

Device-side strategy (per core), v2 — all 16-bit data is fp16:
  - QT/KT/VT [128(2 heads x 64), 4096] fp16 via PE matmuls.
  - S^T[keys, q] = K Q^T per head, K=64 row-packed: head0 in PE rows
    0-63, head1 in rows 64-127 (tile_position row tiling, concurrent).
  - exp split across TWO engines: ~5/8 of tiles on ScalarE
    (activation Exp from PSUM) and ~3/8 on VectorE via the Schraudolph
    bit trick: int16(S*c1 + c2) reinterpreted as fp16 IS approximately
    exp(S*scale) (one tensor_scalar instruction). Softmax averaging
    keeps the ~2% per-element approx error ~1e-2 on the final output.
  - PV col-tiled: head0 -> PSUM partitions 0-63, head1 -> 64-127 in one
    [128, 512] bank; the two matmuls run concurrently (col tiling).
  - softmax denominators via separate M=1 matmuls (ones^T @ P^T),
    4-way col-tiled quads covering two key blocks per 512-col stream.
    Partials land on PSUM partitions 0/32/64/96; a tiny DMA regroups
    them onto one partition, DVE adds + reciprocal, then a K=1 fp32r
    matmul pair broadcasts 1/den back across partitions.
  - normalize: otn = ps_o * psB (DVE, fp16 out); output projection is
    ONE matmul per 128-query block: K=128 contracts both heads at once
    (lhsT = otn[:, ssl], rhs = Wo rows for both heads), PSUM DMA'd
    straight to DRAM.
"""

import math
import os
import sys

import numpy as np

for _p in ("/opt/trn_rl_repo", "/root/.axon_site/_ro/trn_rl_repo"):
    if os.path.isdir(_p) and _p not in sys.path:
        sys.path.insert(0, _p)

import concourse.bass as bass  # noqa: E402
import concourse.mybir as mybir  # noqa: E402
from concourse import bacc  # noqa: E402
from concourse.bass_utils import run_bass_kernel_spmd  # noqa: E402
from concourse.tile import TileContext  # noqa: E402

B, N, D = 2, 4096, 512
H, DH = 8, 64
P = 128                 # SBUF partitions / token block
KB = N // P             # 32 key blocks
QC = N // 512           # 8 query column blocks of 512
KCH = D // P            # 4 contraction chunks for the projections
SCALE = DH ** -0.5
NCORES = 8

# Schraudolph exp in the fp16 bit domain: int16(x*C1 + C2).view(fp16)
# ~= exp(x * SCALE).  C1 = SCALE * 2^10 / ln 2; C2 = 15*2^10 - corr.
EXP_C1 = SCALE * 1024.0 / math.log(2.0)
EXP_C2 = 15360.0 - 366393.0 / 8192.0

# knobs for test.py
TRACE = False
LAST_RESULT = None

_CACHED_NC = None


def use_dve(qc, kb):
    """Which exp tiles run on VectorE (Schraudolph) vs ScalarE."""
    return ((qc * KB + kb) % 8) in (1, 4, 6)


def build_nc():
    f32 = mybir.dt.float32
    f32r = mybir.dt.float32r
    f16 = mybir.dt.float16
    i16 = mybir.dt.int16
    Exp = mybir.ActivationFunctionType.Exp
    Mult = mybir.AluOpType.mult
    Add = mybir.AluOpType.add

    nc = bacc.Bacc()
    xT = nc.declare_dram_parameter("xT", [D, N], f16, isOutput=False)
    wq = nc.declare_dram_parameter("wq", [D, P], f16, isOutput=False)
    wk = nc.declare_dram_parameter("wk", [D, P], f16, isOutput=False)
    wv = nc.declare_dram_parameter("wv", [D, P], f16, isOutput=False)
    wop_d = nc.declare_dram_parameter("wop", [P, D], f16, isOutput=False)
    ident_d = nc.declare_dram_parameter("ident", [P, P], f16, isOutput=False)
    onesf_d = nc.declare_dram_parameter("onesf", [P, DH], f16, isOutput=False)
    hsel_d = nc.declare_dram_parameter("hsel", [2, P], f16, isOutput=False)
    y = nc.declare_dram_parameter("y", [N, D], f32, isOutput=True)

    with TileContext(nc) as tc:
        with (
            tc.tile_pool(name="persist", bufs=1) as persist,
            tc.tile_pool(name="work", bufs=3) as work,
            tc.tile_pool(name="otnp", bufs=2) as otnp,
            tc.tile_pool(name="ysbp", bufs=2) as ysbp,
            tc.tile_pool(name="ptp", bufs=10) as ptp,
            tc.tile_pool(name="ps_big", bufs=2, space="PSUM") as ps_big,
            tc.tile_pool(name="ps_o", bufs=2, space="PSUM") as ps_op,
            tc.tile_pool(name="ps_den", bufs=1, space="PSUM") as ps_denp,
            tc.tile_pool(name="ps_y", bufs=1, space="PSUM") as ps_yp,
        ):
            # ---------------- prologue: loads ----------------
            xt_sb = persist.tile([P, KCH, N], f16, tag="xt")
            for c in range(KCH):
                for cc in range(4):
                    csl = slice(cc * 1024, (cc + 1) * 1024)
                    nc.sync.dma_start(
                        out=xt_sb[:, c, csl], in_=xT[c * P:(c + 1) * P, csl]
                    )

            wq_sb = persist.tile([P, KCH, P], f16, tag="wq")
            wk_sb = persist.tile([P, KCH, P], f16, tag="wk")
            wv_sb = persist.tile([P, KCH, P], f16, tag="wv")
            for w_sb, w_d in ((wq_sb, wq), (wk_sb, wk), (wv_sb, wv)):
                nc.sync.dma_start(
                    out=w_sb, in_=w_d.rearrange("(c p) m -> p c m", p=P)
                )
            wop = persist.tile([P, D], f16, tag="wop")
            nc.sync.dma_start(out=wop, in_=wop_d[:, :])
            ident = persist.tile([P, P], f16, tag="ident")
            nc.sync.dma_start(out=ident, in_=ident_d[:, :])
            onesf = persist.tile([P, DH], f16, tag="onesf")
            nc.sync.dma_start(out=onesf, in_=onesf_d[:, :])
            hsel = persist.tile([2, P], f16, tag="hsel")
            nc.sync.dma_start(out=hsel, in_=hsel_d[:, :])

            qt = persist.tile([P, N], f16, tag="qt")
            kt = persist.tile([P, N], f16, tag="kt")
            vt = persist.tile([P, N], f16, tag="vt")
            v2aug = persist.tile([P, KB, 2, DH + 1], f16, tag="vaug")
            nc.sync.dma_start(
                out=v2aug[:, :, :, DH:DH + 1],
                in_=onesf_d[:, :].rearrange("p (a b c) -> p a b c", a=KB, b=2),
            )

            # ---------------- projections ----------------
            ncopy = [0]

            def cast_copy(dst, src):
                if ncopy[0] % 2 == 0:
                    nc.vector.tensor_copy(dst, src)
                else:
                    nc.scalar.copy(dst, src)
                ncopy[0] += 1

            for dst, w_sb in ((kt, wk_sb), (qt, wq_sb), (vt, wv_sb)):
                for cp in range(4):
                    psp = ps_big.tile([P, 1024], f32, tag="psS")
                    for half in range(2):
                        col = cp * 2 + half
                        csl = slice(col * 512, (col + 1) * 512)
                        for c in range(KCH):
                            nc.tensor.matmul(
                                psp[:, half * 512:(half + 1) * 512],
                                lhsT=w_sb[:, c, :],
                                rhs=xt_sb[:, c, csl],
                                start=(c == 0),
                                stop=(c == KCH - 1),
                            )
                    cast_copy(dst[:, cp * 1024:(cp + 1) * 1024], psp)

            # V natural layout [keys, dims] + ones column via PE transpose
            for batch in range(8):
                psT4 = ps_big.tile([P, 4, P], f16, tag="psS")
                for t in range(4):
                    kb = batch * 4 + t
                    nc.tensor.transpose(
                        psT4[:, t, :], vt[:, kb * P:(kb + 1) * P], ident
                    )
                nc.vector.tensor_copy(
                    v2aug[:, batch * 4:(batch + 1) * 4, :, 0:DH],
                    psT4.rearrange("p a (h d) -> p a h d", h=2),
                )

            # ---------------- attention + output projection ----------------
            state = {}

            def emit_step(qc, kb):
                """S^T for both heads (row-packed) + exp -> pt fp16."""
                ksl = slice(kb * P, (kb + 1) * P)
                qsl = slice(qc * 512, (qc + 1) * 512)
                ps_s = ps_big.tile([P, 1024], f32, tag="psS")
                nc.tensor.matmul(
                    ps_s[:, 0:512], lhsT=kt[0:DH, ksl], rhs=qt[0:DH, qsl]
                )
                nc.tensor.matmul(
                    ps_s[:, 512:1024], lhsT=kt[DH:P, ksl], rhs=qt[DH:P, qsl]
                )
                pt = ptp.tile([P, 1024], f16, tag="pt")
                if use_dve(qc, kb):
                    nc.vector.tensor_scalar(
                        out=pt[:, :].bitcast(i16),
                        in0=ps_s,
                        scalar1=float(EXP_C1),
                        scalar2=float(EXP_C2),
                        op0=Mult,
                        op1=Add,
                    )
                else:
                    nc.scalar.activation(pt, ps_s, func=Exp, scale=SCALE)
                state[(qc, kb)] = pt

            def emit_pv(qc, kb):
                """Col-tiled PV pair: head0 -> partitions 0-63, head1 64-127."""
                pt = state[(qc, kb)]
                if kb == 0:
                    state[(qc, "o")] = ps_op.tile([P, 512], f32, tag="psO", name="pso")
                pso = state[(qc, "o")]
                for h in range(2):
                    nc.tensor.matmul(
                        pso[h * DH:(h + 1) * DH, :],
                        lhsT=v2aug[:, kb, h, 0:DH],
                        rhs=pt[:, h * 512:(h + 1) * 512],
                        start=(kb == 0),
                        stop=(kb == KB - 1),
                        skip_group_check=True,
                    )

            def emit_den(qc, ka):
                """4-way col-tiled denominator quad for key blocks ka, ka+1."""
                if ka == 0:
                    state[(qc, "den")] = ps_denp.tile([P, 512], f32, tag="psD", name="psd")
                psd = state[(qc, "den")]
                for j, (kk, h) in enumerate(
                    ((ka, 0), (ka, 1), (ka + 1, 0), (ka + 1, 1))
                ):
                    pt = state[(qc, kk)]
                    nc.tensor.matmul(
                        psd[32 * j:32 * j + 1, :],
                        lhsT=onesf[:, 0:1],
                        rhs=pt[:, h * 512:(h + 1) * 512],
                        start=(kk <= 1),
                        stop=(kk >= KB - 2),
                        skip_group_check=True,
                        tile_position=(0, 32 * j),
                    )
                state.pop((qc, ka))
                state.pop((qc, ka + 1))

            def emit_den_dma(qc):
                # drain den partials to SBUF, then DMA-regroup partitions
                # {0,32,64,96} -> {0,0,1,1} (engines cannot move data across
                # partitions; DMA cannot read PSUM)
                psd = state.pop((qc, "den"))
                dfull = work.tile([P, 512], f32, tag="wk", name="dfull")
                nc.vector.tensor_copy(dfull, psd)
                den_sb = work.tile([2, 2, 512], f32, tag="wk", name="densb")
                for j in range(4):
                    nc.sync.dma_start(
                        out=den_sb[j // 2:j // 2 + 1, j % 2, :],
                        in_=dfull[32 * j:32 * j + 1, :],
                    )
                state[(qc, "dsb")] = den_sb

            def emit_add(qc):
                den_sb = state.pop((qc, "dsb"))
                den2 = work.tile([2, 512], f32, tag="wk", name="den2")
                nc.vector.tensor_add(
                    den2, den_sb[:, 0, :], den_sb[:, 1, :]
                )
                state[(qc, "d2")] = den2

            def emit_recip(qc):
                den2 = state.pop((qc, "d2"))
                dinv32 = work.tile([2, 512], f32, tag="wk", name="dinv32")
                nc.vector.reciprocal_approx_fast(out=dinv32, in_=den2)
                state[(qc, "d32")] = dinv32

            def emit_cvt(qc):
                dinv32 = state.pop((qc, "d32"))
                deninv = work.tile([2, 512], f16, tag="wk", name="dinv16")
                nc.vector.tensor_copy(deninv, dinv32)
                state[(qc, "dinv")] = deninv

            def emit_bcast(qc):
                # one K=2 matmul: hsel row-selects each head's 1/den row
                # into partitions 0-63 / 64-127
                deninv = state.pop((qc, "dinv"))
                psB = ps_yp.tile([P, 512], f32, tag="psY")
                nc.tensor.matmul(
                    psB, lhsT=hsel, rhs=deninv, skip_group_check=True
                )
                state[(qc, "psB")] = psB

            def emit_bcopy(qc):
                # engines read at most one PSUM operand; stage psB in SBUF
                psB = state.pop((qc, "psB"))
                binv = work.tile([P, 512], f32, tag="wk", name="binv")
                cast_copy(binv, psB)
                state[(qc, "binv")] = binv

            def emit_mul(qc):
                binv = state.pop((qc, "binv"))
                pso = state.pop((qc, "o"))
                otn = otnp.tile([P, 512], f16, tag="otn")
                nc.vector.tensor_mul(otn, pso, binv)
                state[(qc, "otn")] = otn

            def emit_proj(qc, sub):
                # one matmul contracts both heads: K=128 (2h x 64 dims)
                otn = state[(qc, "otn")]
                psY = ps_yp.tile([P, 512], f32, tag="psY")
                nc.tensor.matmul(
                    psY, lhsT=otn[:, sub * P:(sub + 1) * P], rhs=wop
                )
                ysb = ysbp.tile([P, 512], f32, tag="ysb")
                cast_copy(ysb, psY)
                r0 = qc * 512 + sub * P
                nc.sync.dma_start(out=y[r0:r0 + P, :], in_=ysb)
                if sub == 3:
                    state.pop((qc, "otn"))

            def emit_tail(qc, kb):
                """Tail work for qc, paced across early kb slots of qc+1."""
                if kb == 0:
                    emit_pv(qc, KB - 4)
                    emit_pv(qc, KB - 3)
                elif kb == 2:
                    emit_pv(qc, KB - 2)
                    emit_pv(qc, KB - 1)
                elif kb == 4:
                    emit_den(qc, KB - 8)
                elif kb == 5:
                    emit_den(qc, KB - 6)
                elif kb == 6:
                    emit_den(qc, KB - 4)
                elif kb == 7:
                    emit_den(qc, KB - 2)
                elif kb == 8:
                    emit_den_dma(qc)
                elif kb == 9:
                    emit_add(qc)
                elif kb == 10:
                    emit_recip(qc)
                elif kb == 11:
                    emit_cvt(qc)
                elif kb == 12:
                    emit_bcast(qc)
                elif kb == 13:
                    emit_bcopy(qc)
                elif kb == 14:
                    emit_mul(qc)
                elif kb in (16, 20, 24, 28):
                    emit_proj(qc, (kb - 16) // 4)

            # 2-step groups: same-kind matmul slots adjacent (S^T,S^T then
            # PV,PV then one den quad) — same-config slots chain ~216ns on
            # the PE vs ~320ns at every config switch.
            for qc in range(QC):
                for g in range(KB // 2):
                    t = 2 * g
                    emit_step(qc, t)
                    emit_step(qc, t + 1)
                    if qc > 0:
                        emit_tail(qc - 1, t)
                        emit_tail(qc - 1, t + 1)
                    if t >= 4:
                        emit_pv(qc, t - 4)
                        emit_pv(qc, t - 3)
                    if t >= 8:
                        emit_den(qc, t - 8)
            for kb in range(29):
                emit_tail(QC - 1, kb)

    if not nc.is_finalized():
        nc.finalize()
    return nc


def _get_nc():
    global _CACHED_NC
    if _CACHED_NC is None:
        _CACHED_NC = build_nc()
    return _CACHED_NC


def make_in_maps(x, Wq, Wk, Wv, Wo):
    f16 = np.float16
    in_maps = []
    for c in range(NCORES):
        b, p = c // 4, c % 4
        cols = slice(p * P, (p + 1) * P)
        in_maps.append({
            "xT": np.ascontiguousarray(x[b].T).astype(f16),
            "wq": np.ascontiguousarray(Wq[:, cols]).astype(f16),
            "wk": np.ascontiguousarray(Wk[:, cols]).astype(f16),
            "wv": np.ascontiguousarray(Wv[:, cols]).astype(f16),
            "wop": np.ascontiguousarray(Wo[cols, :]).astype(f16),
            "ident": np.eye(P, dtype=f16),
            "onesf": np.ones((P, DH), dtype=f16),
            "hsel": np.repeat(np.eye(2, dtype=f16), DH, axis=1),
        })
    return in_maps


def kernel(x, Wq, Wk, Wv, Wo, bo):
    global LAST_RESULT
    x = np.asarray(x, dtype=np.float32)
    Wq = np.asarray(Wq, dtype=np.float32)
    Wk = np.asarray(Wk, dtype=np.float32)
    Wv = np.asarray(Wv, dtype=np.float32)
    Wo = np.asarray(Wo, dtype=np.float32)
    bo = np.asarray(bo, dtype=np.float32)

    in_maps = make_in_maps(x, Wq, Wk, Wv, Wo)
    nc = _get_nc()
    res = run_bass_kernel_spmd(nc, in_maps, list(range(NCORES)), trace=TRACE)
    LAST_RESULT = res

    out = np.zeros((B, N, D), dtype=np.float32)
    for c in range(NCORES):
        out[c // 4] += res.results[c]["y"]
    out += bo[None, None, :]
    return out


# revision 13
# speedup vs baseline: 1.1009x; 1.1009x over previous
"""Trainium2 Bass kernel for CrossAttention (self-attention) nn module.

Reference computation (B=2, N=4096, D=512, H=8, DH=64):
    q, k, v = x@Wq, x@Wk, x@Wv          # [B, N, 512]
    per head: S = q k^T / sqrt(64); P = softmax(S); O = P v
    out = concat_heads(O) @ Wo + bo     # [B, N, 512]

Sharding: batch*head-pair across 8 cores. Core c handles batch c//4 and
head pair c%4 (heads 2p, 2p+1). Each core computes its two heads'
attention plus its partial output projection O_pair @ Wo[rows]; the host
sums the four partials per batch and adds the bias.

Device-side strategy (per core), v2 — all 16-bit data is fp16:
  - QT/KT/VT [128(2 heads x 64), 4096] fp16 via PE matmuls.
  - S^T[keys, q] = K Q^T per head, K=64 row-packed: head0 in PE rows
    0-63, head1 in rows 64-127 (tile_position row tiling, concurrent).
  - exp split across TWO engines: ~5/8 of tiles on ScalarE
    (activation Exp from PSUM) and ~3/8 on VectorE via the Schraudolph
    bit trick: int16(S*c1 + c2) reinterpreted as fp16 IS approximately
    exp(S*scale) (one tensor_scalar instruction). Softmax averaging
    keeps the ~2% per-element approx error ~1e-2 on the final output.
  - PV col-tiled: head0 -> PSUM partitions 0-63, head1 -> 64-127 in one
    [128, 512] bank; the two matmuls run concurrently (col tiling).
  - softmax denominators via separate M=1 matmuls (ones^T @ P^T),
    4-way col-tiled quads covering two key blocks per 512-col stream.
    Partials land on PSUM partitions 0/32/64/96; a tiny DMA regroups
    them onto one partition, DVE adds + reciprocal, then a K=1 fp32r
    matmul pair broadcasts 1/den back across partitions.
  - normalize: otn = ps_o * psB (DVE, fp16 out); output projection is
    ONE matmul per 128-query block: K=128 contracts both heads at once
    (lhsT = otn[:, ssl], rhs = Wo rows for both heads), PSUM DMA'd
    straight to DRAM.
"""

import math
import os
import sys

import numpy as np

for _p in ("/opt/trn_rl_repo", "/root/.axon_site/_ro/trn_rl_repo"):
    if os.path.isdir(_p) and _p not in sys.path:
        sys.path.insert(0, _p)

import concourse.bass as bass  # noqa: E402
import concourse.mybir as mybir  # noqa: E402
from concourse import bacc  # noqa: E402
from concourse.bass_utils import run_bass_kernel_spmd  # noqa: E402
from concourse.tile import TileContext  # noqa: E402

B, N, D = 2, 4096, 512
H, DH = 8, 64
P = 128                 # SBUF partitions / token block
KB = N // P             # 32 key blocks
QC = N // 512           # 8 query column blocks of 512
KCH = D // P            # 4 contraction chunks for the projections
SCALE = DH ** -0.5
NCORES = 8

# Schraudolph exp in the fp16 bit domain: int16(x*C1 + C2).view(fp16)
# ~= exp(x * SCALE).  C1 = SCALE * 2^10 / ln 2; C2 = 15*2^10 - corr.
EXP_C1 = SCALE * 1024.0 / math.log(2.0)
EXP_C2 = 15360.0 - 366393.0 / 8192.0

# knobs for test.py
TRACE = False
LAST_RESULT = None

_CACHED_NC = None


def use_dve(qc, kb):
    """Which exp tiles run on VectorE (Schraudolph) vs ScalarE."""
    return ((qc * KB + kb) % 8) in (1, 4, 6)


def build_nc():
    f32 = mybir.dt.float32
    f32r = mybir.dt.float32r
    f16 = mybir.dt.float16
    i16 = mybir.dt.int16
    Exp = mybir.ActivationFunctionType.Exp
    Mult = mybir.AluOpType.mult
    Add = mybir.AluOpType.add

    nc = bacc.Bacc()
    xT = nc.declare_dram_parameter("xT", [D, N], f16, isOutput=False)
    wq = nc.declare_dram_parameter("wq", [D, P], f16, isOutput=False)
    wk = nc.declare_dram_parameter("wk", [D, P], f16, isOutput=False)
    wv = nc.declare_dram_parameter("wv", [D, P], f16, isOutput=False)
    wop_d = nc.declare_dram_parameter("wop", [P, D], f16, isOutput=False)
    ident_d = nc.declare_dram_parameter("ident", [P, P], f16, isOutput=False)
    onesf_d = nc.declare_dram_parameter("onesf", [P, DH], f16, isOutput=False)
    hsel_d = nc.declare_dram_parameter("hsel", [2, P], f16, isOutput=False)
    y = nc.declare_dram_parameter("y", [N, D], f32, isOutput=True)

    with TileContext(nc) as tc:
        with (
            tc.tile_pool(name="persist", bufs=1) as persist,
            tc.tile_pool(name="work", bufs=3) as work,
            tc.tile_pool(name="otnp", bufs=2) as otnp,
            tc.tile_pool(name="ysbp", bufs=2) as ysbp,
            tc.tile_pool(name="ptp", bufs=15) as ptp,
            tc.tile_pool(name="ps_big", bufs=2, space="PSUM") as ps_big,
            tc.tile_pool(name="ps_o", bufs=2, space="PSUM") as ps_op,
            tc.tile_pool(name="ps_den", bufs=1, space="PSUM") as ps_denp,
            tc.tile_pool(name="ps_y", bufs=1, space="PSUM") as ps_yp,
        ):
            # ---------------- prologue: loads ----------------
            xt_sb = persist.tile([P, KCH, N], f16, tag="xt")
            for c in range(KCH):
                for cc in range(4):
                    csl = slice(cc * 1024, (cc + 1) * 1024)
                    nc.sync.dma_start(
                        out=xt_sb[:, c, csl], in_=xT[c * P:(c + 1) * P, csl]
                    )

            wq_sb = persist.tile([P, KCH, P], f16, tag="wq")
            wk_sb = persist.tile([P, KCH, P], f16, tag="wk")
            wv_sb = persist.tile([P, KCH, P], f16, tag="wv")
            for w_sb, w_d in ((wq_sb, wq), (wk_sb, wk), (wv_sb, wv)):
                nc.sync.dma_start(
                    out=w_sb, in_=w_d.rearrange("(c p) m -> p c m", p=P)
                )
            wop = persist.tile([P, D], f16, tag="wop")
            nc.sync.dma_start(out=wop, in_=wop_d[:, :])
            ident = persist.tile([P, P], f16, tag="ident")
            nc.sync.dma_start(out=ident, in_=ident_d[:, :])
            onesf = persist.tile([P, DH], f16, tag="onesf")
            nc.sync.dma_start(out=onesf, in_=onesf_d[:, :])
            hsel = persist.tile([2, P], f16, tag="hsel")
            nc.sync.dma_start(out=hsel, in_=hsel_d[:, :])

            qt = persist.tile([P, N], f16, tag="qt")
            kt = persist.tile([P, N], f16, tag="kt")
            vt = persist.tile([P, N], f16, tag="vt")
            v2aug = persist.tile([P, KB, 2, DH + 1], f16, tag="vaug")
            nc.sync.dma_start(
                out=v2aug[:, :, :, DH:DH + 1],
                in_=onesf_d[:, :].rearrange("p (a b c) -> p a b c", a=KB, b=2),
            )

            # ---------------- projections ----------------
            ncopy = [0]

            def cast_copy(dst, src):
                if ncopy[0] % 2 == 0:
                    nc.vector.tensor_copy(dst, src)
                else:
                    nc.scalar.copy(dst, src)
                ncopy[0] += 1

            for dst, w_sb in ((kt, wk_sb), (qt, wq_sb), (vt, wv_sb)):
                for cp in range(4):
                    psp = ps_big.tile([P, 1024], f32, tag="psS")
                    for half in range(2):
                        col = cp * 2 + half
                        csl = slice(col * 512, (col + 1) * 512)
                        for c in range(KCH):
                            nc.tensor.matmul(
                                psp[:, half * 512:(half + 1) * 512],
                                lhsT=w_sb[:, c, :],
                                rhs=xt_sb[:, c, csl],
                                start=(c == 0),
                                stop=(c == KCH - 1),
                            )
                    cast_copy(dst[:, cp * 1024:(cp + 1) * 1024], psp)

            # V natural layout [keys, dims] + ones column via PE transpose
            for batch in range(8):
                psT4 = ps_big.tile([P, 4, P], f16, tag="psS")
                for t in range(4):
                    kb = batch * 4 + t
                    nc.tensor.transpose(
                        psT4[:, t, :], vt[:, kb * P:(kb + 1) * P], ident
                    )
                nc.vector.tensor_copy(
                    v2aug[:, batch * 4:(batch + 1) * 4, :, 0:DH],
                    psT4.rearrange("p a (h d) -> p a h d", h=2),
                )

            # ---------------- attention + output projection ----------------
            state = {}

            def emit_step(qc, kb):
                """S^T for both heads (row-packed) + exp -> pt fp16."""
                ksl = slice(kb * P, (kb + 1) * P)
                qsl = slice(qc * 512, (qc + 1) * 512)
                ps_s = ps_big.tile([P, 1024], f32, tag="psS")
                nc.tensor.matmul(
                    ps_s[:, 0:512], lhsT=kt[0:DH, ksl], rhs=qt[0:DH, qsl]
                )
                nc.tensor.matmul(
                    ps_s[:, 512:1024], lhsT=kt[DH:P, ksl], rhs=qt[DH:P, qsl]
                )
                pt = ptp.tile([P, 1024], f16, tag="pt")
                if use_dve(qc, kb):
                    nc.vector.tensor_scalar(
                        out=pt[:, :].bitcast(i16),
                        in0=ps_s,
                        scalar1=float(EXP_C1),
                        scalar2=float(EXP_C2),
                        op0=Mult,
                        op1=Add,
                    )
                else:
                    nc.scalar.activation(pt, ps_s, func=Exp, scale=SCALE)
                state[(qc, kb)] = pt

            def emit_pv(qc, kb):
                """Col-tiled PV pair: head0 -> partitions 0-63, head1 64-127."""
                pt = state[(qc, kb)]
                if kb == 0:
                    state[(qc, "o")] = ps_op.tile([P, 512], f32, tag="psO", name="pso")
                pso = state[(qc, "o")]
                for h in range(2):
                    nc.tensor.matmul(
                        pso[h * DH:(h + 1) * DH, :],
                        lhsT=v2aug[:, kb, h, 0:DH],
                        rhs=pt[:, h * 512:(h + 1) * 512],
                        start=(kb == 0),
                        stop=(kb == KB - 1),
                        skip_group_check=True,
                    )

            def emit_den(qc, ka):
                """4-way col-tiled denominator quad for key blocks ka, ka+1."""
                if ka == 0:
                    state[(qc, "den")] = ps_denp.tile([P, 512], f32, tag="psD", name="psd")
                psd = state[(qc, "den")]
                for j, (kk, h) in enumerate(
                    ((ka, 0), (ka, 1), (ka + 1, 0), (ka + 1, 1))
                ):
                    pt = state[(qc, kk)]
                    nc.tensor.matmul(
                        psd[32 * j:32 * j + 1, :],
                        lhsT=onesf[:, 0:1],
                        rhs=pt[:, h * 512:(h + 1) * 512],
                        start=(kk <= 1),
                        stop=(kk >= KB - 2),
                        skip_group_check=True,
                        tile_position=(0, 32 * j),
                    )
                state.pop((qc, ka))
                state.pop((qc, ka + 1))

            def emit_den_dma(qc):
                # drain den partials to SBUF, then DMA-regroup partitions
                # {0,32,64,96} -> {0,0,1,1} (engines cannot move data across
                # partitions; DMA cannot read PSUM)
                psd = state.pop((qc, "den"))
                dfull = work.tile([P, 512], f32, tag="wk", name="dfull")
                nc.vector.tensor_copy(dfull, psd)
                den_sb = work.tile([2, 2, 512], f32, tag="wk", name="densb")
                for j in range(4):
                    nc.sync.dma_start(
                        out=den_sb[j % 2:j % 2 + 1, j // 2, :],
                        in_=dfull[32 * j:32 * j + 1, :],
                    )
                state[(qc, "dsb")] = den_sb

            def emit_add(qc):
                den_sb = state.pop((qc, "dsb"))
                den2 = work.tile([2, 512], f32, tag="wk", name="den2")
                nc.vector.tensor_add(
                    den2, den_sb[:, 0, :], den_sb[:, 1, :]
                )
                state[(qc, "d2")] = den2

            def emit_recip(qc):
                den2 = state.pop((qc, "d2"))
                dinv32 = work.tile([2, 512], f32, tag="wk", name="dinv32")
                nc.vector.reciprocal_approx_fast(out=dinv32, in_=den2)
                state[(qc, "d32")] = dinv32

            def emit_cvt(qc):
                dinv32 = state.pop((qc, "d32"))
                deninv = work.tile([2, 512], f16, tag="wk", name="dinv16")
                nc.vector.tensor_copy(deninv, dinv32)
                state[(qc, "dinv")] = deninv

            def emit_bcast(qc):
                # one K=2 matmul: hsel row-selects each head's 1/den row
                # into partitions 0-63 / 64-127
                deninv = state.pop((qc, "dinv"))
                psB = ps_yp.tile([P, 512], f32, tag="psY")
                nc.tensor.matmul(
                    psB, lhsT=hsel, rhs=deninv, skip_group_check=True
                )
                state[(qc, "psB")] = psB

            def emit_bcopy(qc):
                # engines read at most one PSUM operand; stage psB in SBUF
                psB = state.pop((qc, "psB"))
                binv = work.tile([P, 512], f32, tag="wk", name="binv")
                cast_copy(binv, psB)
                state[(qc, "binv")] = binv

            def emit_mul(qc):
                binv = state.pop((qc, "binv"))
                pso = state.pop((qc, "o"))
                otn = otnp.tile([P, 512], f16, tag="otn")
                nc.vector.tensor_mul(otn, pso, binv)
                state[(qc, "otn")] = otn

            def emit_proj(qc, sub):
                # one matmul contracts both heads: K=128 (2h x 64 dims)
                otn = state[(qc, "otn")]
                psY = ps_yp.tile([P, 512], f32, tag="psY")
                nc.tensor.matmul(
                    psY, lhsT=otn[:, sub * P:(sub + 1) * P], rhs=wop
                )
                ysb = ysbp.tile([P, 512], f32, tag="ysb")
                cast_copy(ysb, psY)
                r0 = qc * 512 + sub * P
                nc.sync.dma_start(out=y[r0:r0 + P, :], in_=ysb)
                if sub == 3:
                    state.pop((qc, "otn"))

            def emit_tail(qc, kb):
                """Tail work for qc, paced across early kb slots of qc+1.
                Generous gaps keep the PE from ever waiting on the DVE/DMA
                denominator chain (a PE idle window re-throttles HAM)."""
                if kb == 0:
                    emit_pv(qc, KB - 4)
                    emit_pv(qc, KB - 3)
                elif kb == 1:
                    emit_pv(qc, KB - 2)
                    emit_pv(qc, KB - 1)
                elif 2 <= kb <= 7:
                    emit_den(qc, KB - 12 + 2 * (kb - 2))
                elif kb == 8:
                    emit_den_dma(qc)
                elif kb == 10:
                    emit_add(qc)
                elif kb == 12:
                    emit_recip(qc)
                elif kb == 14:
                    emit_cvt(qc)
                elif kb == 16:
                    emit_bcast(qc)
                elif kb == 17:
                    emit_bcopy(qc)
                elif kb == 19:
                    emit_mul(qc)
                elif kb in (21, 24, 27, 30):
                    emit_proj(qc, {21: 0, 24: 1, 27: 2, 30: 3}[kb])

            # 2-step groups: same-kind matmul slots adjacent (S^T,S^T then
            # PV,PV then one den quad) — same-config slots chain ~216ns on
            # the PE vs ~320ns at every config switch.
            for qc in range(QC):
                for g in range(KB // 2):
                    t = 2 * g
                    emit_step(qc, t)
                    emit_step(qc, t + 1)
                    if qc > 0:
                        emit_tail(qc - 1, t)
                        emit_tail(qc - 1, t + 1)
                    if t >= 4:
                        emit_pv(qc, t - 4)
                        emit_pv(qc, t - 3)
                    if t >= 12:
                        emit_den(qc, t - 12)
            for kb in range(31):
                emit_tail(QC - 1, kb)

    if not nc.is_finalized():
        nc.finalize()
    return nc


def _get_nc():
    global _CACHED_NC
    if _CACHED_NC is None:
        _CACHED_NC = build_nc()
    return _CACHED_NC


def make_in_maps(x, Wq, Wk, Wv, Wo):
    f16 = np.float16
    in_maps = []
    for c in range(NCORES):
        b, p = c // 4, c % 4
        cols = slice(p * P, (p + 1) * P)
        in_maps.append({
            "xT": np.ascontiguousarray(x[b].T).astype(f16),
            "wq": np.ascontiguousarray(Wq[:, cols]).astype(f16),
            "wk": np.ascontiguousarray(Wk[:, cols]).astype(f16),
            "wv": np.ascontiguousarray(Wv[:, cols]).astype(f16),
            "wop": np.ascontiguousarray(Wo[cols, :]).astype(f16),
            "ident": np.eye(P, dtype=f16),
            "onesf": np.ones((P, DH), dtype=f16),
            "hsel": np.repeat(np.eye(2, dtype=f16), DH, axis=1),
        })
    return in_maps


def kernel(x, Wq, Wk, Wv, Wo, bo):
    global LAST_RESULT
    x = np.asarray(x, dtype=np.float32)
    Wq = np.asarray(Wq, dtype=np.float32)
    Wk = np.asarray(Wk, dtype=np.float32)
    Wv = np.asarray(Wv, dtype=np.float32)
    Wo = np.asarray(Wo, dtype=np.float32)
    bo = np.asarray(bo, dtype=np.float32)

    in_maps = make_in_maps(x, Wq, Wk, Wv, Wo)
    nc = _get_nc()
    res = run_bass_kernel_spmd(nc, in_maps, list(range(NCORES)), trace=TRACE)
    LAST_RESULT = res

    out = np.zeros((B, N, D), dtype=np.float32)
    for c in range(NCORES):
        out[c // 4] += res.results[c]["y"]
    out += bo[None, None, :]
    return out


# revision 14
# speedup vs baseline: 1.1363x; 1.0321x over previous
"""Trainium2 Bass kernel for CrossAttention (self-attention) nn module.

Reference computation (B=2, N=4096, D=512, H=8, DH=64):
    q, k, v = x@Wq, x@Wk, x@Wv          # [B, N, 512]
    per head: S = q k^T / sqrt(64); P = softmax(S); O = P v
    out = concat_heads(O) @ Wo + bo     # [B, N, 512]

Sharding: batch*head-pair across 8 cores. Core c handles batch c//4 and
head pair c%4 (heads 2p, 2p+1). Each core computes its two heads'
attention plus its partial output projection O_pair @ Wo[rows]; the host
sums the four partials per batch and adds the bias.

Device-side strategy (per core), v2 — all 16-bit data is fp16:
  - QT/KT/VT [128(2 heads x 64), 4096] fp16 via PE matmuls.
  - S^T[keys, q] = K Q^T per head, K=64 row-packed: head0 in PE rows
    0-63, head1 in rows 64-127 (tile_position row tiling, concurrent).
  - exp split across TWO engines: ~5/8 of tiles on ScalarE
    (activation Exp from PSUM) and ~3/8 on VectorE via the Schraudolph
    bit trick: int16(S*c1 + c2) reinterpreted as fp16 IS approximately
    exp(S*scale) (one tensor_scalar instruction). Softmax averaging
    keeps the ~2% per-element approx error ~1e-2 on the final output.
  - PV col-tiled: head0 -> PSUM partitions 0-63, head1 -> 64-127 in one
    [128, 512] bank; the two matmuls run concurrently (col tiling).
  - softmax denominators via separate M=1 matmuls (ones^T @ P^T),
    4-way col-tiled quads covering two key blocks per 512-col stream.
    Partials land on PSUM partitions 0/32/64/96; a tiny DMA regroups
    them onto one partition, DVE adds + reciprocal, then a K=1 fp32r
    matmul pair broadcasts 1/den back across partitions.
  - normalize: otn = ps_o * psB (DVE, fp16 out); output projection is
    ONE matmul per 128-query block: K=128 contracts both heads at once
    (lhsT = otn[:, ssl], rhs = Wo rows for both heads), PSUM DMA'd
    straight to DRAM.
"""

import math
import os
import sys

import numpy as np

for _p in ("/opt/trn_rl_repo", "/root/.axon_site/_ro/trn_rl_repo"):
    if os.path.isdir(_p) and _p not in sys.path:
        sys.path.insert(0, _p)

import concourse.bass as bass  # noqa: E402
import concourse.mybir as mybir  # noqa: E402
from concourse import bacc  # noqa: E402
from concourse.bass_utils import run_bass_kernel_spmd  # noqa: E402
from concourse.tile import TileContext  # noqa: E402

B, N, D = 2, 4096, 512
H, DH = 8, 64
P = 128                 # SBUF partitions / token block
KB = N // P             # 32 key blocks
QC = N // 512           # 8 query column blocks of 512
KCH = D // P            # 4 contraction chunks for the projections
SCALE = DH ** -0.5
NCORES = 8

# Schraudolph exp in the fp16 bit domain: int16(x*C1 + C2).view(fp16)
# ~= exp(x * SCALE).  C1 = SCALE * 2^10 / ln 2; C2 = 15*2^10 - corr.
EXP_C1 = SCALE * 1024.0 / math.log(2.0)
EXP_C2 = 15360.0 - 366393.0 / 8192.0

# knobs for test.py
TRACE = False
LAST_RESULT = None

_CACHED_NC = None


def use_dve(qc, kb):
    """Which exp tiles run on VectorE (Schraudolph) vs ScalarE."""
    return ((qc * KB + kb) % 8) in (1, 4, 6)


def build_nc():
    f32 = mybir.dt.float32
    f32r = mybir.dt.float32r
    f16 = mybir.dt.float16
    i16 = mybir.dt.int16
    Exp = mybir.ActivationFunctionType.Exp
    Mult = mybir.AluOpType.mult
    Add = mybir.AluOpType.add

    nc = bacc.Bacc()
    xT = nc.declare_dram_parameter("xT", [D, N], f16, isOutput=False)
    wq = nc.declare_dram_parameter("wq", [D, P], f16, isOutput=False)
    wk = nc.declare_dram_parameter("wk", [D, P], f16, isOutput=False)
    wv = nc.declare_dram_parameter("wv", [D, P], f16, isOutput=False)
    wop_d = nc.declare_dram_parameter("wop", [P, D], f16, isOutput=False)
    ident_d = nc.declare_dram_parameter("ident", [P, P], f16, isOutput=False)
    onesf_d = nc.declare_dram_parameter("onesf", [P, DH], f16, isOutput=False)
    hsel_d = nc.declare_dram_parameter("hsel", [2, P], f16, isOutput=False)
    y = nc.declare_dram_parameter("y", [N, D], f32, isOutput=True)

    with TileContext(nc) as tc:
        with (
            tc.tile_pool(name="persist", bufs=1) as persist,
            tc.tile_pool(name="work", bufs=3) as work,
            tc.tile_pool(name="otnp", bufs=2) as otnp,
            tc.tile_pool(name="ysbp", bufs=2) as ysbp,
            tc.tile_pool(name="ptp", bufs=15) as ptp,
            tc.tile_pool(name="ps_big", bufs=4, space="PSUM") as ps_big,
            tc.tile_pool(name="ps_o", bufs=2, space="PSUM") as ps_op,
            tc.tile_pool(name="ps_den", bufs=1, space="PSUM") as ps_denp,
            tc.tile_pool(name="ps_y", bufs=1, space="PSUM") as ps_yp,
        ):
            # ---------------- prologue: loads ----------------
            # small weight DMAs first; xt in cc-major order so the first
            # projection column-pair's inputs land earliest
            xt_sb = persist.tile([P, KCH, N], f16, tag="xt")
            wq_sb = persist.tile([P, KCH, P], f16, tag="wq")
            wk_sb = persist.tile([P, KCH, P], f16, tag="wk")
            wv_sb = persist.tile([P, KCH, P], f16, tag="wv")
            for w_sb, w_d in ((wq_sb, wq), (wk_sb, wk), (wv_sb, wv)):
                nc.sync.dma_start(
                    out=w_sb, in_=w_d.rearrange("(c p) m -> p c m", p=P)
                )
            wop = persist.tile([P, D], f16, tag="wop")
            nc.sync.dma_start(out=wop, in_=wop_d[:, :])
            ident = persist.tile([P, P], f16, tag="ident")
            nc.sync.dma_start(out=ident, in_=ident_d[:, :])
            onesf = persist.tile([P, DH], f16, tag="onesf")
            nc.sync.dma_start(out=onesf, in_=onesf_d[:, :])
            hsel = persist.tile([2, P], f16, tag="hsel")
            nc.sync.dma_start(out=hsel, in_=hsel_d[:, :])

            qt = persist.tile([P, N], f16, tag="qt")
            kt = persist.tile([P, N], f16, tag="kt")
            vt = persist.tile([P, N], f16, tag="vt")
            v2aug = persist.tile([P, KB, 2, DH + 1], f16, tag="vaug")
            nc.sync.dma_start(
                out=v2aug[:, :, :, DH:DH + 1],
                in_=onesf_d[:, :].rearrange("p (a b c) -> p a b c", a=KB, b=2),
            )
            for cc in range(4):
                csl = slice(cc * 1024, (cc + 1) * 1024)
                for c in range(KCH):
                    nc.sync.dma_start(
                        out=xt_sb[:, c, csl], in_=xT[c * P:(c + 1) * P, csl]
                    )

            # ---------------- projections ----------------
            ncopy = [0]

            def cast_copy(dst, src):
                if ncopy[0] % 2 == 0:
                    nc.vector.tensor_copy(dst, src)
                else:
                    nc.scalar.copy(dst, src)
                ncopy[0] += 1

            for dst, w_sb in ((kt, wk_sb), (qt, wq_sb), (vt, wv_sb)):
                for col in range(8):
                    psp = ps_big.tile([P, 512], f32, tag="psS", name="psp")
                    csl = slice(col * 512, (col + 1) * 512)
                    for c in range(KCH):
                        nc.tensor.matmul(
                            psp,
                            lhsT=w_sb[:, c, :],
                            rhs=xt_sb[:, c, csl],
                            start=(c == 0),
                            stop=(c == KCH - 1),
                        )
                    cast_copy(dst[:, csl], psp)

            # V natural layout [keys, dims] + ones column via PE transpose
            for batch in range(8):
                psT4 = ps_big.tile([P, 4, P], f16, tag="psS")
                for t in range(4):
                    kb = batch * 4 + t
                    nc.tensor.transpose(
                        psT4[:, t, :], vt[:, kb * P:(kb + 1) * P], ident
                    )
                nc.vector.tensor_copy(
                    v2aug[:, batch * 4:(batch + 1) * 4, :, 0:DH],
                    psT4.rearrange("p a (h d) -> p a h d", h=2),
                )

            # ---------------- attention + output projection ----------------
            state = {}

            nhalf = [0]

            def emit_step(qc, kb):
                """S^T for both heads (row-packed, separate PSUM banks) +
                per-half exp -> pt fp16.  Half-tile granularity halves the
                ps_s recycle latency (S^T(t+2) waits only one half-exp) and
                lets ScalarE and VectorE chew the same step concurrently."""
                ksl = slice(kb * P, (kb + 1) * P)
                qsl = slice(qc * 512, (qc + 1) * 512)
                pa = ps_big.tile([P, 512], f32, tag="psS", name="psa")
                pb = ps_big.tile([P, 512], f32, tag="psS", name="psb")
                nc.tensor.matmul(pa, lhsT=kt[0:DH, ksl], rhs=qt[0:DH, qsl])
                nc.tensor.matmul(pb, lhsT=kt[DH:P, ksl], rhs=qt[DH:P, qsl])
                pt = ptp.tile([P, 1024], f16, tag="pt")
                for h, src in ((0, pa), (1, pb)):
                    dst = pt[:, h * 512:(h + 1) * 512]
                    if nhalf[0] % 16 in (1, 3, 6, 8, 10, 13, 15):
                        nc.vector.tensor_scalar(
                            out=dst.bitcast(i16),
                            in0=src,
                            scalar1=float(EXP_C1),
                            scalar2=float(EXP_C2),
                            op0=Mult,
                            op1=Add,
                        )
                    else:
                        nc.scalar.activation(dst, src, func=Exp, scale=SCALE)
                    nhalf[0] += 1
                state[(qc, kb)] = pt

            def emit_pv(qc, kb):
                """Col-tiled PV pair: head0 -> partitions 0-63, head1 64-127."""
                pt = state[(qc, kb)]
                if kb == 0:
                    state[(qc, "o")] = ps_op.tile([P, 512], f32, tag="psO", name="pso")
                pso = state[(qc, "o")]
                for h in range(2):
                    nc.tensor.matmul(
                        pso[h * DH:(h + 1) * DH, :],
                        lhsT=v2aug[:, kb, h, 0:DH],
                        rhs=pt[:, h * 512:(h + 1) * 512],
                        start=(kb == 0),
                        stop=(kb == KB - 1),
                        skip_group_check=True,
                    )

            def emit_den(qc, ka):
                """4-way col-tiled denominator quad for key blocks ka, ka+1."""
                if ka == 0:
                    state[(qc, "den")] = ps_denp.tile([P, 512], f32, tag="psD", name="psd")
                psd = state[(qc, "den")]
                for j, (kk, h) in enumerate(
                    ((ka, 0), (ka, 1), (ka + 1, 0), (ka + 1, 1))
                ):
                    pt = state[(qc, kk)]
                    nc.tensor.matmul(
                        psd[32 * j:32 * j + 1, :],
                        lhsT=onesf[:, 0:1],
                        rhs=pt[:, h * 512:(h + 1) * 512],
                        start=(kk <= 1),
                        stop=(kk >= KB - 2),
                        skip_group_check=True,
                        tile_position=(0, 32 * j),
                    )
                state.pop((qc, ka))
                state.pop((qc, ka + 1))

            def emit_den_dma(qc):
                # drain den partials to SBUF, then DMA-regroup partitions
                # {0,32,64,96} -> {0,0,1,1} (engines cannot move data across
                # partitions; DMA cannot read PSUM)
                psd = state.pop((qc, "den"))
                dfull = work.tile([P, 512], f32, tag="wk", name="dfull")
                nc.vector.tensor_copy(dfull, psd)
                den_sb = work.tile([2, 2, 512], f32, tag="wk", name="densb")
                for j in range(4):
                    nc.sync.dma_start(
                        out=den_sb[j % 2:j % 2 + 1, j // 2, :],
                        in_=dfull[32 * j:32 * j + 1, :],
                    )
                state[(qc, "dsb")] = den_sb

            def emit_add(qc):
                den_sb = state.pop((qc, "dsb"))
                den2 = work.tile([2, 512], f32, tag="wk", name="den2")
                nc.vector.tensor_add(
                    den2, den_sb[:, 0, :], den_sb[:, 1, :]
                )
                state[(qc, "d2")] = den2

            def emit_recip(qc):
                den2 = state.pop((qc, "d2"))
                dinv32 = work.tile([2, 512], f32, tag="wk", name="dinv32")
                nc.vector.reciprocal_approx_fast(out=dinv32, in_=den2)
                state[(qc, "d32")] = dinv32

            def emit_cvt(qc):
                dinv32 = state.pop((qc, "d32"))
                deninv = work.tile([2, 512], f16, tag="wk", name="dinv16")
                nc.vector.tensor_copy(deninv, dinv32)
                state[(qc, "dinv")] = deninv

            def emit_bcast(qc):
                # one K=2 matmul: hsel row-selects each head's 1/den row
                # into partitions 0-63 / 64-127
                deninv = state.pop((qc, "dinv"))
                psB = ps_yp.tile([P, 512], f32, tag="psY")
                nc.tensor.matmul(
                    psB, lhsT=hsel, rhs=deninv, skip_group_check=True
                )
                state[(qc, "psB")] = psB

            def emit_bcopy(qc):
                # engines read at most one PSUM operand; stage psB in SBUF
                psB = state.pop((qc, "psB"))
                binv = work.tile([P, 512], f32, tag="wk", name="binv")
                cast_copy(binv, psB)
                state[(qc, "binv")] = binv

            def emit_mul(qc):
                binv = state.pop((qc, "binv"))
                pso = state.pop((qc, "o"))
                otn = otnp.tile([P, 512], f16, tag="otn")
                nc.vector.tensor_mul(otn, pso, binv)
                state[(qc, "otn")] = otn

            def emit_proj(qc, sub):
                # one matmul contracts both heads: K=128 (2h x 64 dims)
                otn = state[(qc, "otn")]
                psY = ps_yp.tile([P, 512], f32, tag="psY")
                nc.tensor.matmul(
                    psY, lhsT=otn[:, sub * P:(sub + 1) * P], rhs=wop
                )
                ysb = ysbp.tile([P, 512], f32, tag="ysb")
                cast_copy(ysb, psY)
                r0 = qc * 512 + sub * P
                nc.sync.dma_start(out=y[r0:r0 + P, :], in_=ysb)
                if sub == 3:
                    state.pop((qc, "otn"))

            def emit_tail(qc, kb):
                """Tail work for qc, paced across early kb slots of qc+1.
                Generous gaps keep the PE from ever waiting on the DVE/DMA
                denominator chain (a PE idle window re-throttles HAM)."""
                if kb == 0:
                    emit_pv(qc, KB - 4)
                    emit_pv(qc, KB - 3)
                elif kb == 1:
                    emit_pv(qc, KB - 2)
                    emit_pv(qc, KB - 1)
                elif 2 <= kb <= 7:
                    emit_den(qc, KB - 12 + 2 * (kb - 2))
                elif kb == 8:
                    emit_den_dma(qc)
                elif kb == 10:
                    emit_add(qc)
                elif kb == 12:
                    emit_recip(qc)
                elif kb == 14:
                    emit_cvt(qc)
                elif kb == 16:
                    emit_bcast(qc)
                elif kb == 17:
                    emit_bcopy(qc)
                elif kb == 19:
                    emit_mul(qc)
                elif kb in (21, 24, 27, 30):
                    emit_proj(qc, {21: 0, 24: 1, 27: 2, 30: 3}[kb])

            # 2-step groups: same-kind matmul slots adjacent (S^T,S^T then
            # PV,PV then one den quad) — same-config slots chain ~216ns on
            # the PE vs ~320ns at every config switch.
            for qc in range(QC):
                for g in range(KB // 2):
                    t = 2 * g
                    emit_step(qc, t)
                    emit_step(qc, t + 1)
                    if qc > 0:
                        emit_tail(qc - 1, t)
                        emit_tail(qc - 1, t + 1)
                    if t >= 4:
                        emit_pv(qc, t - 4)
                        emit_pv(qc, t - 3)
                    if t >= 12:
                        emit_den(qc, t - 12)
            for kb in range(31):
                emit_tail(QC - 1, kb)

    if not nc.is_finalized():
        nc.finalize()
    return nc


def _get_nc():
    global _CACHED_NC
    if _CACHED_NC is None:
        _CACHED_NC = build_nc()
    return _CACHED_NC


def make_in_maps(x, Wq, Wk, Wv, Wo):
    f16 = np.float16
    in_maps = []
    for c in range(NCORES):
        b, p = c // 4, c % 4
        cols = slice(p * P, (p + 1) * P)
        in_maps.append({
            "xT": np.ascontiguousarray(x[b].T).astype(f16),
            "wq": np.ascontiguousarray(Wq[:, cols]).astype(f16),
            "wk": np.ascontiguousarray(Wk[:, cols]).astype(f16),
            "wv": np.ascontiguousarray(Wv[:, cols]).astype(f16),
            "wop": np.ascontiguousarray(Wo[cols, :]).astype(f16),
            "ident": np.eye(P, dtype=f16),
            "onesf": np.ones((P, DH), dtype=f16),
            "hsel": np.repeat(np.eye(2, dtype=f16), DH, axis=1),
        })
    return in_maps


def kernel(x, Wq, Wk, Wv, Wo, bo):
    global LAST_RESULT
    x = np.asarray(x, dtype=np.float32)
    Wq = np.asarray(Wq, dtype=np.float32)
    Wk = np.asarray(Wk, dtype=np.float32)
    Wv = np.asarray(Wv, dtype=np.float32)
    Wo = np.asarray(Wo, dtype=np.float32)
    bo = np.asarray(bo, dtype=np.float32)

    in_maps = make_in_maps(x, Wq, Wk, Wv, Wo)
    nc = _get_nc()
    res = run_bass_kernel_spmd(nc, in_maps, list(range(NCORES)), trace=TRACE)
    LAST_RESULT = res

    out = np.zeros((B, N, D), dtype=np.float32)
    for c in range(NCORES):
        out[c // 4] += res.results[c]["y"]
    out += bo[None, None, :]
    return out


# revision 16
# speedup vs baseline: 1.1856x; 1.0434x over previous
"""Trainium2 Bass kernel for CrossAttention (self-attention) nn module.

Reference computation (B=2, N=4096, D=512, H=8, DH=64):
    q, k, v = x@Wq, x@Wk, x@Wv          # [B, N, 512]
    per head: S = q k^T / sqrt(64); P = softmax(S); O = P v
    out = concat_heads(O) @ Wo + bo     # [B, N, 512]

Sharding: batch*head-pair across 8 cores. Core c handles batch c//4 and
head pair c%4 (heads 2p, 2p+1). Each core computes its two heads'
attention plus its partial output projection O_pair @ Wo[rows]; the host
sums the four partials per batch and adds the bias.

Device-side strategy (per core), v2 — all 16-bit data is fp16:
  - QT/KT/VT [128(2 heads x 64), 4096] fp16 via PE matmuls.
  - S^T[keys, q] = K Q^T per head, K=64 row-packed: head0 in PE rows
    0-63, head1 in rows 64-127 (tile_position row tiling, concurrent).
  - exp split across TWO engines: ~5/8 of tiles on ScalarE
    (activation Exp from PSUM) and ~3/8 on VectorE via the Schraudolph
    bit trick: int16(S*c1 + c2) reinterpreted as fp16 IS approximately
    exp(S*scale) (one tensor_scalar instruction). Softmax averaging
    keeps the ~2% per-element approx error ~1e-2 on the final output.
  - PV col-tiled: head0 -> PSUM partitions 0-63, head1 -> 64-127 in one
    [128, 512] bank; the two matmuls run concurrently (col tiling).
  - softmax denominators via separate M=1 matmuls (ones^T @ P^T),
    4-way col-tiled quads covering two key blocks per 512-col stream.
    Partials land on PSUM partitions 0/32/64/96; a tiny DMA regroups
    them onto one partition, DVE adds + reciprocal, then a K=1 fp32r
    matmul pair broadcasts 1/den back across partitions.
  - normalize: otn = ps_o * psB (DVE, fp16 out); output projection is
    ONE matmul per 128-query block: K=128 contracts both heads at once
    (lhsT = otn[:, ssl], rhs = Wo rows for both heads), PSUM DMA'd
    straight to DRAM.
"""

import math
import os
import sys

import numpy as np

for _p in ("/opt/trn_rl_repo", "/root/.axon_site/_ro/trn_rl_repo"):
    if os.path.isdir(_p) and _p not in sys.path:
        sys.path.insert(0, _p)

import concourse.bass as bass  # noqa: E402
import concourse.mybir as mybir  # noqa: E402
from concourse import bacc  # noqa: E402
from concourse.bass_utils import run_bass_kernel_spmd  # noqa: E402
from concourse.tile import TileContext  # noqa: E402

B, N, D = 2, 4096, 512
H, DH = 8, 64
P = 128                 # SBUF partitions / token block
KB = N // P             # 32 key blocks
QC = N // 512           # 8 query column blocks of 512
KCH = D // P            # 4 contraction chunks for the projections
SCALE = DH ** -0.5
NCORES = 8

# Schraudolph exp in the fp16 bit domain: int16(x*C1 + C2).view(fp16)
# ~= exp(x * SCALE).  C1 = SCALE * 2^10 / ln 2; C2 = 15*2^10 - corr.
EXP_C1 = SCALE * 1024.0 / math.log(2.0)
EXP_C2 = 15360.0 - 366393.0 / 8192.0

# knobs for test.py
TRACE = False
LAST_RESULT = None

_CACHED_NC = None


def use_dve(qc, kb):
    """Which exp tiles run on VectorE (Schraudolph) vs ScalarE."""
    return ((qc * KB + kb) % 8) in (1, 4, 6)


def build_nc():
    f32 = mybir.dt.float32
    f32r = mybir.dt.float32r
    f16 = mybir.dt.float16
    i16 = mybir.dt.int16
    Exp = mybir.ActivationFunctionType.Exp
    Mult = mybir.AluOpType.mult
    Add = mybir.AluOpType.add

    nc = bacc.Bacc()
    xT = nc.declare_dram_parameter("xT", [D, N], f16, isOutput=False)
    wq = nc.declare_dram_parameter("wq", [D, P], f16, isOutput=False)
    wk = nc.declare_dram_parameter("wk", [D, P], f16, isOutput=False)
    wv = nc.declare_dram_parameter("wv", [D, P], f16, isOutput=False)
    wop_d = nc.declare_dram_parameter("wop", [P, D], f16, isOutput=False)
    ident_d = nc.declare_dram_parameter("ident", [P, P], f16, isOutput=False)
    onesf_d = nc.declare_dram_parameter("onesf", [P, DH], f16, isOutput=False)
    hsel_d = nc.declare_dram_parameter("hsel", [2, P], f32, isOutput=False)
    y = nc.declare_dram_parameter("y", [N, D], f32, isOutput=True)

    with TileContext(nc) as tc:
        with (
            tc.tile_pool(name="persist", bufs=1) as persist,
            tc.tile_pool(name="work", bufs=3) as work,
            tc.tile_pool(name="otnp", bufs=2) as otnp,
            tc.tile_pool(name="ysbp", bufs=2) as ysbp,
            tc.tile_pool(name="ptp", bufs=15) as ptp,
            tc.tile_pool(name="ps_big", bufs=4, space="PSUM") as ps_big,
            tc.tile_pool(name="ps_o", bufs=2, space="PSUM") as ps_op,
            tc.tile_pool(name="ps_den", bufs=1, space="PSUM") as ps_denp,
            tc.tile_pool(name="ps_y", bufs=1, space="PSUM") as ps_yp,
        ):
            # ---------------- prologue: loads ----------------
            # small weight DMAs first; xt in cc-major order so the first
            # projection column-pair's inputs land earliest
            xt_sb = persist.tile([P, KCH, N], f16, tag="xt")
            wq_sb = persist.tile([P, KCH, P], f16, tag="wq")
            wk_sb = persist.tile([P, KCH, P], f16, tag="wk")
            wv_sb = persist.tile([P, KCH, P], f16, tag="wv")
            for w_sb, w_d in ((wq_sb, wq), (wk_sb, wk), (wv_sb, wv)):
                nc.sync.dma_start(
                    out=w_sb, in_=w_d.rearrange("(c p) m -> p c m", p=P)
                )
            wop = persist.tile([P, D], f16, tag="wop")
            nc.sync.dma_start(out=wop, in_=wop_d[:, :])
            ident = persist.tile([P, P], f16, tag="ident")
            nc.sync.dma_start(out=ident, in_=ident_d[:, :])
            onesf = persist.tile([P, DH], f16, tag="onesf")
            nc.sync.dma_start(out=onesf, in_=onesf_d[:, :])
            hsel = persist.tile([2, P], f32, tag="hsel")
            nc.sync.dma_start(out=hsel, in_=hsel_d[:, :])

            qt = persist.tile([P, N], f16, tag="qt")
            kt = persist.tile([P, N], f16, tag="kt")
            vt = persist.tile([P, N], f16, tag="vt")
            v2aug = persist.tile([P, KB, 2, DH + 1], f16, tag="vaug")
            nc.vector.memset(v2aug[:, :, :, DH:DH + 1], 1.0)
            for cc in range(4):
                csl = slice(cc * 1024, (cc + 1) * 1024)
                for c in range(KCH):
                    nc.sync.dma_start(
                        out=xt_sb[:, c, csl], in_=xT[c * P:(c + 1) * P, csl]
                    )

            # ---------------- projections ----------------
            ncopy = [0]

            def cast_copy(dst, src):
                if ncopy[0] % 2 == 0:
                    nc.vector.tensor_copy(dst, src)
                else:
                    nc.scalar.copy(dst, src)
                ncopy[0] += 1

            for dst, w_sb in ((kt, wk_sb), (qt, wq_sb), (vt, wv_sb)):
                for col in range(8):
                    psp = ps_big.tile([P, 512], f32, tag="psS", name="psp")
                    csl = slice(col * 512, (col + 1) * 512)
                    for c in range(KCH):
                        nc.tensor.matmul(
                            psp,
                            lhsT=w_sb[:, c, :],
                            rhs=xt_sb[:, c, csl],
                            start=(c == 0),
                            stop=(c == KCH - 1),
                        )
                    cast_copy(dst[:, csl], psp)

            # V natural layout [keys, dims] + ones column via PE transpose
            for batch in range(8):
                psT4 = ps_big.tile([P, 4, P], f16, tag="psS")
                for t in range(4):
                    kb = batch * 4 + t
                    nc.tensor.transpose(
                        psT4[:, t, :], vt[:, kb * P:(kb + 1) * P], ident
                    )
                nc.vector.tensor_copy(
                    v2aug[:, batch * 4:(batch + 1) * 4, :, 0:DH],
                    psT4.rearrange("p a (h d) -> p a h d", h=2),
                )

            # ---------------- attention + output projection ----------------
            state = {}

            nhalf = [0]

            def emit_step(qc, kb):
                """S^T for both heads (row-packed, separate PSUM banks) +
                per-half exp -> pt fp16.  Half-tile granularity halves the
                ps_s recycle latency (S^T(t+2) waits only one half-exp) and
                lets ScalarE and VectorE chew the same step concurrently."""
                ksl = slice(kb * P, (kb + 1) * P)
                qsl = slice(qc * 512, (qc + 1) * 512)
                pa = ps_big.tile([P, 512], f32, tag="psS", name="psa")
                pb = ps_big.tile([P, 512], f32, tag="psS", name="psb")
                nc.tensor.matmul(pa, lhsT=kt[0:DH, ksl], rhs=qt[0:DH, qsl])
                nc.tensor.matmul(pb, lhsT=kt[DH:P, ksl], rhs=qt[DH:P, qsl])
                pt = ptp.tile([P, 1024], f16, tag="pt")
                for h, src in ((0, pa), (1, pb)):
                    dst = pt[:, h * 512:(h + 1) * 512]
                    if nhalf[0] % 16 in (1, 3, 6, 8, 10, 13, 15):
                        nc.vector.tensor_scalar(
                            out=dst.bitcast(i16),
                            in0=src,
                            scalar1=float(EXP_C1),
                            scalar2=float(EXP_C2),
                            op0=Mult,
                            op1=Add,
                        )
                    else:
                        nc.scalar.activation(dst, src, func=Exp, scale=SCALE)
                    nhalf[0] += 1
                state[(qc, kb)] = pt

            def emit_pv(qc, kb):
                """Col-tiled PV pair: head0 -> partitions 0-63, head1 64-127."""
                pt = state[(qc, kb)]
                if kb == 0:
                    state[(qc, "o")] = ps_op.tile([P, 512], f32, tag="psO", name="pso")
                pso = state[(qc, "o")]
                for h in range(2):
                    nc.tensor.matmul(
                        pso[h * DH:(h + 1) * DH, :],
                        lhsT=v2aug[:, kb, h, 0:DH],
                        rhs=pt[:, h * 512:(h + 1) * 512],
                        start=(kb == 0),
                        stop=(kb == KB - 1),
                        skip_group_check=True,
                    )

            def emit_den(qc, ka):
                """4-way col-tiled denominator quad for key blocks ka, ka+1."""
                if ka == 0:
                    state[(qc, "den")] = ps_denp.tile([P, 512], f32, tag="psD", name="psd")
                psd = state[(qc, "den")]
                for j, (kk, h) in enumerate(
                    ((ka, 0), (ka, 1), (ka + 1, 0), (ka + 1, 1))
                ):
                    pt = state[(qc, kk)]
                    nc.tensor.matmul(
                        psd[32 * j:32 * j + 1, :],
                        lhsT=onesf[:, 0:1],
                        rhs=pt[:, h * 512:(h + 1) * 512],
                        start=(kk <= 1),
                        stop=(kk >= KB - 2),
                        skip_group_check=True,
                        tile_position=(0, 32 * j),
                    )
                state.pop((qc, ka))
                state.pop((qc, ka + 1))

            def emit_den_dma(qc):
                # drain den partials to SBUF, then DMA-regroup partitions
                # {0,32,64,96} -> {0,0,1,1} (engines cannot move data across
                # partitions; DMA cannot read PSUM)
                psd = state.pop((qc, "den"))
                dfull = work.tile([P, 512], f32, tag="wk", name="dfull")
                nc.vector.tensor_copy(dfull, psd)
                den_sb = work.tile([2, 2, 512], f32, tag="wk", name="densb")
                for s in range(2):
                    nc.sync.dma_start(
                        out=den_sb[:, s, :],
                        in_=dfull[64 * s:64 * s + 33:32, :],
                    )
                state[(qc, "dsb")] = den_sb

            def emit_add(qc):
                den_sb = state.pop((qc, "dsb"))
                den2 = work.tile([2, 512], f32, tag="wk", name="den2")
                nc.vector.tensor_add(
                    den2, den_sb[:, 0, :], den_sb[:, 1, :]
                )
                state[(qc, "d2")] = den2

            def emit_recip(qc):
                den2 = state.pop((qc, "d2"))
                deninv = work.tile([2, 512], f32, tag="wk", name="dinv")
                nc.vector.reciprocal_approx_fast(out=deninv, in_=den2)
                state[(qc, "dinv")] = deninv

            def emit_bcast(qc):
                # one K=2 matmul: hsel row-selects each head's 1/den row
                # into partitions 0-63 / 64-127
                deninv = state.pop((qc, "dinv"))
                psB = ps_yp.tile([P, 512], f32, tag="psY")
                nc.tensor.matmul(
                    psB, lhsT=hsel, rhs=deninv, skip_group_check=True
                )
                state[(qc, "psB")] = psB

            def emit_bcopy(qc):
                # engines read at most one PSUM operand; stage psB in SBUF
                psB = state.pop((qc, "psB"))
                binv = work.tile([P, 512], f32, tag="wk", name="binv")
                cast_copy(binv, psB)
                state[(qc, "binv")] = binv

            def emit_mul(qc):
                binv = state.pop((qc, "binv"))
                pso = state.pop((qc, "o"))
                otn = otnp.tile([P, 512], f16, tag="otn")
                nc.vector.tensor_mul(otn, pso, binv)
                state[(qc, "otn")] = otn

            def emit_proj(qc, sub):
                # one matmul contracts both heads: K=128 (2h x 64 dims)
                otn = state[(qc, "otn")]
                psY = ps_yp.tile([P, 512], f32, tag="psY")
                nc.tensor.matmul(
                    psY, lhsT=otn[:, sub * P:(sub + 1) * P], rhs=wop
                )
                ysb = ysbp.tile([P, 512], f32, tag="ysb")
                cast_copy(ysb, psY)
                r0 = qc * 512 + sub * P
                nc.sync.dma_start(out=y[r0:r0 + P, :], in_=ysb)
                if sub == 3:
                    state.pop((qc, "otn"))

            def emit_tail(qc, kb):
                """Tail work for qc, paced across early kb slots of qc+1.
                Generous gaps keep the PE from ever waiting on the DVE/DMA
                denominator chain (a PE idle window re-throttles HAM)."""
                if kb == 0:
                    emit_pv(qc, KB - 4)
                    emit_pv(qc, KB - 3)
                elif kb == 1:
                    emit_pv(qc, KB - 2)
                    emit_pv(qc, KB - 1)
                elif 2 <= kb <= 7:
                    emit_den(qc, KB - 12 + 2 * (kb - 2))
                elif kb == 8:
                    emit_den_dma(qc)
                elif kb == 10:
                    emit_add(qc)
                elif kb == 12:
                    emit_recip(qc)
                elif kb == 15:
                    emit_bcast(qc)
                elif kb == 17:
                    emit_bcopy(qc)
                elif kb == 19:
                    emit_mul(qc)
                elif kb in (21, 24, 27, 30):
                    emit_proj(qc, {21: 0, 24: 1, 27: 2, 30: 3}[kb])

            # 2-step groups: same-kind matmul slots adjacent (S^T,S^T then
            # PV,PV then one den quad) — same-config slots chain ~216ns on
            # the PE vs ~320ns at every config switch.
            for qc in range(QC):
                for g in range(KB // 2):
                    t = 2 * g
                    emit_step(qc, t)
                    emit_step(qc, t + 1)
                    if qc > 0:
                        emit_tail(qc - 1, t)
                        emit_tail(qc - 1, t + 1)
                    if t >= 4:
                        emit_pv(qc, t - 4)
                        emit_pv(qc, t - 3)
                    if t >= 12:
                        emit_den(qc, t - 12)
            for kb in range(31):
                emit_tail(QC - 1, kb)

    if not nc.is_finalized():
        nc.finalize()
    return nc


def _get_nc():
    global _CACHED_NC
    if _CACHED_NC is None:
        _CACHED_NC = build_nc()
    return _CACHED_NC


def make_in_maps(x, Wq, Wk, Wv, Wo):
    f16 = np.float16
    in_maps = []
    for c in range(NCORES):
        b, p = c // 4, c % 4
        cols = slice(p * P, (p + 1) * P)
        in_maps.append({
            "xT": np.ascontiguousarray(x[b].T).astype(f16),
            "wq": np.ascontiguousarray(Wq[:, cols]).astype(f16),
            "wk": np.ascontiguousarray(Wk[:, cols]).astype(f16),
            "wv": np.ascontiguousarray(Wv[:, cols]).astype(f16),
            "wop": np.ascontiguousarray(Wo[cols, :]).astype(f16),
            "ident": np.eye(P, dtype=f16),
            "onesf": np.ones((P, DH), dtype=f16),
            "hsel": np.repeat(np.eye(2, dtype=np.float32), DH, axis=1),
        })
    return in_maps


def kernel(x, Wq, Wk, Wv, Wo, bo):
    global LAST_RESULT
    x = np.asarray(x, dtype=np.float32)
    Wq = np.asarray(Wq, dtype=np.float32)
    Wk = np.asarray(Wk, dtype=np.float32)
    Wv = np.asarray(Wv, dtype=np.float32)
    Wo = np.asarray(Wo, dtype=np.float32)
    bo = np.asarray(bo, dtype=np.float32)

    in_maps = make_in_maps(x, Wq, Wk, Wv, Wo)
    nc = _get_nc()
    res = run_bass_kernel_spmd(nc, in_maps, list(range(NCORES)), trace=TRACE)
    LAST_RESULT = res

    out = np.zeros((B, N, D), dtype=np.float32)
    for c in range(NCORES):
        out[c // 4] += res.results[c]["y"]
    out += bo[None, None, :]
    return out
